# revision 39
# baseline (speedup 1.0000x reference)
"""Trainium2 Bass kernel for CrossModalAttention.

Problem: B=8, C=512, H=W=48 (S=2304 spatial), TC=512.
  tf = w_tp @ text + b_tp           (1x1 conv)
  Q  = w_q @ img + b_q ;  K = w_k @ tf + b_k ;  V = w_v @ tf + b_v
  attn = softmax_t(scale * Q^T K)   (full spatial cross attention)
  out  = img + attn @ V^T

Sharding: data-parallel over batch — one batch element per NeuronCore (8 cores).

Per-core device kernel (all big matmuls in float32r: full PE speed at N>=256,
~tf32 precision, fp32 PSUM accumulation):
  - the text projection is FOLDED on the host: K = (w_k w_tp) text + (w_k b_tp
    + b_k), V likewise => the tf intermediate never exists on device; K and
    V_T stream directly from text chunks.
  - weights are pre-transposed on host to [c_in, c_out] so no on-device
    transposes are needed anywhere.
  - `scale` is folded into w_q/b_q on the host (exact: softmax input identical).
  - scores are computed TRANSPOSED: st[t, s] = sum_o K[o,t] Q'[o,s] so that
    V_T[t, o] (computed directly transposed by swapping matmul operand roles)
    can consume exp(st) with no transposes. Softmax is computed without max
    subtraction (|scores| <= ~30 for this problem => exp is fp32-safe).
  - softmax denominator: VectorE tree-adds the 18 exp tiles (DVE is otherwise
    idle) and one tiny fp32 N=2 matmul per s-slice reduces over partitions;
    normalization is a per-partition reciprocal scale fused into the PSUM->SBUF
    copy of the output.
  - K's bias is applied on device (free per-partition ACT bias, and exactly
    cancels in softmax anyway); V's effective bias is added on the host
    (softmax rows sum to 1 so it contributes exactly +b_v_eff to out).
  - the kernel returns O_T[s, o]; host transposes and adds img_feat + b_v_eff.
  - text and img are DMA-streamed per 512-column chunk (chunk-major contiguous
    layout) over both HWDGE rings; weights go over the gpsimd SWDGE ring; a
    short N=512 warmup matmul burst holds the PE HAM clock at 2.4 GHz through
    the DMA-limited start.
"""

import numpy as np

B, C, S = 8, 512, 2304
H = W = 48
P = 128
CB = C // P     # 4 channel blocks
TB = S // P     # 18 key/value position blocks
# s-chunk widths (>=256 keeps float32r matmul at 1 cycle/row)
# small chunk first: only 512KB of text must land before the first matmul
CHUNKS = [(2048, 256), (0, 512), (512, 512), (1024, 512), (1536, 512)]

_CACHE = {}


def _build():
    """Trace + compile the per-core Bass kernel. Returns the compiled Bacc."""
    import concourse.bacc as bacc
    import concourse.tile as tile
    import concourse.mybir as mybir

    f32 = mybir.dt.float32
    f32r = mybir.dt.float32r
    AF = mybir.ActivationFunctionType

    nc = bacc.Bacc("TRN2", target_bir_lowering=False, debug=False)

    # chunk-major layout: [chunk, c_block, partition, 512] so each per-chunk
    # DMA is one fully contiguous 256KB block (max DMA bandwidth)
    NCH = len(CHUNKS)
    img_d = nc.dram_tensor("img", [NCH, CB, P, 512], f32r, kind="ExternalInput")
    text_d = nc.dram_tensor("text", [NCH, CB, P, 512], f32r, kind="ExternalInput")
    wq_d = nc.dram_tensor("wq", [CB, P, C], f32r, kind="ExternalInput")
    wk_d = nc.dram_tensor("wk", [CB, P, C], f32r, kind="ExternalInput")
    wv_d = nc.dram_tensor("wv", [CB, P, C], f32r, kind="ExternalInput")
    bq_d = nc.dram_tensor("bq", [CB, P, 1], f32, kind="ExternalInput")
    bk_d = nc.dram_tensor("bk", [CB, P, 1], f32, kind="ExternalInput")
    ones_d = nc.dram_tensor("ones", [P, 2], f32, kind="ExternalInput")
    out_d = nc.dram_tensor("out", [TB, P, C], f32, kind="ExternalOutput")

    # queue mode: later pools get fresh (ring) addresses instead of
    # immediately reusing earlier space => streaming DMAs don't WAR-stall
    with tile.TileContext(nc, pool_alloc_mode="queue") as tc:
        with (
            tc.tile_pool(name="pp", bufs=2, space="PSUM") as pp,
            tc.tile_pool(name="psc", bufs=3, space="PSUM") as psc,
            tc.tile_pool(name="po", bufs=3, space="PSUM") as po,
            # persistent SBUF
            tc.tile_pool(name="const", bufs=1) as constp,
            tc.tile_pool(name="wts", bufs=1) as wts,
            tc.tile_pool(name="kpool", bufs=1) as kpool,
            tc.tile_pool(name="vpool", bufs=1) as vpool,
        ):
            # PE warmup: full-width (N=512) matmuls on a memset tile keep the
            # PE array genuinely busy while the first text chunk DMAs in, so
            # the HAM clock gate reaches 8/8 (2.4 GHz) before real work starts.
            wu = constp.tile([P, 512], f32, name="wu", tag="wu")
            nc.vector.memset(wu, 0.0)
            wur = constp.tile([P, 512], f32r, name="wur", tag="wur")
            nc.scalar.activation(wur, wu, mybir.ActivationFunctionType.Copy)
            for i in range(24):
                acc_w = pp.tile([P, 512], f32, name="acc_w", tag="pp")
                nc.tensor.matmul(acc_w[:, :512], wur[:, :P], wur, start=True, stop=True)

            # const tiles; DMAs emitted after the first text chunk (their
            # issue latency must not delay the first streamed transfer)
            ones32 = constp.tile([P, 2], f32, name="ones32", tag="ones32")
            bq = [constp.tile([P, 1], f32, name=f"bq{c}", tag=f"bq{c}") for c in range(CB)]
            bk = [constp.tile([P, 1], f32, name=f"bk{c}", tag=f"bk{c}") for c in range(CB)]

            def load_consts():
                for c in range(CB):
                    nc.gpsimd.dma_start(bk[c], bk_d[c])
                    nc.gpsimd.dma_start(bq[c], bq_d[c])
                nc.gpsimd.dma_start(ones32, ones_d[:, :])

            # K and V weights are needed within the first few us of phase 1:
            # put them at the head of the two fast HWDGE rings (wk on sync,
            # wv on scalar), ahead of the text stream; wq/biases go over the
            # slower gpsimd SWDGE ring (needed much later)
            # wk split across all three DMA paths (first K group needs all 4
            # blocks); wv follows on SWDGE, landing before the first V_T group
            wk, wv, wq = [], [], []
            wk_eng = [nc.sync, nc.scalar, nc.sync, nc.scalar]
            for c in range(CB):
                t = wts.tile([P, C], f32r, name=f"wk{c}", tag=f"wk{c}")
                wk_eng[c].dma_start(t, wk_d[c])
                wk.append(t)
                wv.append(wts.tile([P, C], f32r, name=f"wv{c}", tag=f"wv{c}"))
                wq.append(wts.tile([P, C], f32r, name=f"wq{c}", tag=f"wq{c}"))
            for c in range(CB):
                nc.gpsimd.dma_start(wv[c], wv_d[c])

            K = [kpool.tile([P, S], f32r, name=f"k{c}", tag=f"k{c}") for c in range(CB)]
            VT = [vpool.tile([P, C], f32r, name=f"vt{t}", tag=f"vt{t}") for t in range(TB)]
            Qf = [kpool.tile([P, S], f32r, name=f"qf{o}", tag=f"qf{o}") for o in range(CB)]

            imgp_cm = tc.tile_pool(name="imgp", bufs=2)
            imgp = imgp_cm.__enter__()
            img_tiles = {}

            def load_img_chunk(ci, cw):
                tiles = []
                for c in range(CB):
                    it = imgp.tile([P, 512], f32r, name=f"imgc{c}", tag=f"imgc{c}")
                    eng = nc.sync if c % 2 == 0 else nc.scalar
                    eng.dma_start(it[:, :cw], img_d[ci, c][:, :cw])
                    tiles.append(it)
                img_tiles[ci] = tiles

            # ---- phase 1: K, V_T and Q, all streamed ----
            # K[o, t] = sum_c wk[c, o] text[c, t]  (+ b_k_eff)
            # V_T[t, o] = sum_c text[c, t] wv[c, o]  (b_v_eff added on host)
            # Q'[o, s] = sum_c wq[c, o] img[c, s]   (+ b_q_eff), one chunk
            # DELAYED so it acts as PE filler while the next text chunk lands
            def q_group(cj):
                s0j, cwj = CHUNKS[cj]
                img_c = img_tiles.pop(cj)
                for o in range(CB):
                    acc = pp.tile([P, 512], f32, name="acc_q", tag="pp")
                    for c in range(CB):
                        nc.tensor.matmul(
                            acc[:, :cwj],
                            wq[c][:, o * P:(o + 1) * P],
                            img_c[c][:, :cwj],
                            start=(c == 0), stop=(c == CB - 1),
                        )
                    nc.vector.tensor_scalar_add(
                        Qf[o][:, s0j:s0j + cwj], acc[:, :cwj], bq[o])

            with tc.tile_pool(name="textp", bufs=3) as textp:
                for ci, (s0, cw) in enumerate(CHUNKS):
                    text_c = []
                    for c in range(CB):
                        tt = textp.tile([P, 512], f32r, name=f"text{c}", tag=f"text{c}")
                        # split each chunk across both HWDGE rings (sync + scalar)
                        eng = nc.sync if c % 2 == 0 else nc.scalar
                        eng.dma_start(tt[:, :cw], text_d[ci, c][:, :cw])
                        text_c.append(tt)
                    if ci == 0:
                        load_consts()
                        for c in range(CB):
                            nc.gpsimd.dma_start(wq[c], wq_d[c])
                    else:
                        # img for the PREVIOUS chunk: consumed by q_group(ci-1)
                        # at the end of this iteration; keeps the ring head
                        # clear for wk + the first text chunks
                        load_img_chunk(ci - 1, CHUNKS[ci - 1][1])
                    if ci == 0:
                        # c-outer with 4 interleaved PSUM groups: each text
                        # block is consumed as soon as its DMA lands (the 4
                        # blocks arrive staggered over the rings), so the PE
                        # never idles waiting for the whole chunk
                        accs = [pp.tile([P, 512], f32, name="acc_k", tag="pp"),
                                pp.tile([P, 512], f32, name="acc_k", tag="pp"),
                                psc.tile([P, 512], f32, name="acc_k2", tag="psc"),
                                psc.tile([P, 512], f32, name="acc_k2", tag="psc")]
                        for c in range(CB):
                            for o in range(CB):
                                nc.tensor.matmul(
                                    accs[o][:, :cw],
                                    wk[c][:, o * P:(o + 1) * P],
                                    text_c[c][:, :cw],
                                    start=(c == 0), stop=(c == CB - 1),
                                )
                        for o in range(CB):
                            nc.vector.tensor_scalar_add(
                                K[o][:, s0:s0 + cw], accs[o][:, :cw], bk[o])
                    else:
                        for o in range(CB):
                            acc = pp.tile([P, 512], f32, name="acc_k", tag="pp")
                            for c in range(CB):
                                nc.tensor.matmul(
                                    acc[:, :cw],
                                    wk[c][:, o * P:(o + 1) * P],
                                    text_c[c][:, :cw],
                                    start=(c == 0), stop=(c == CB - 1),
                                )
                            nc.vector.tensor_scalar_add(
                                K[o][:, s0:s0 + cw], acc[:, :cw], bk[o])
                    for tl in range(cw // P):
                        t = s0 // P + tl
                        acc = pp.tile([P, 512], f32, name="acc_v", tag="pp")
                        for c in range(CB):
                            nc.tensor.matmul(
                                acc[:, :C],
                                text_c[c][:, tl * P:(tl + 1) * P],
                                wv[c][:, :C],
                                start=(c == 0), stop=(c == CB - 1),
                            )
                        nc.vector.tensor_copy(VT[t], acc[:, :C])
                    if ci > 0:
                        q_group(ci - 1)
                load_img_chunk(NCH - 1, CHUNKS[NCH - 1][1])
                q_group(NCH - 1)

            # ---- phase 2: attention, chunked over s ----
            with (
                tc.tile_pool(name="ep", bufs=1) as ep,
                tc.tile_pool(name="dsp", bufs=1) as dsp,
                tc.tile_pool(name="outp", bufs=3) as outp,
                tc.tile_pool(name="rp", bufs=2) as rp,
            ):
                for ci, (s0, cw) in enumerate(CHUNKS):
                    # E[t, s] = exp(sum_o K[o,t] Q'[o,s]);
                    # DVE tree-adds E tiles into ds (sum over t-blocks)
                    E = []
                    ds_prev = None
                    for t in range(TB):
                        acc = psc.tile([P, 512], f32, name="acc_sc", tag="psc")
                        for o in range(CB):
                            nc.tensor.matmul(
                                acc[:, :cw],
                                K[o][:, t * P:(t + 1) * P],
                                Qf[o][:, s0:s0 + cw],
                                start=(o == 0), stop=(o == CB - 1),
                            )
                        et = ep.tile([P, 512], f32r, name=f"e{t}", tag=f"e{t}")
                        nc.scalar.activation(et[:, :cw], acc[:, :cw], AF.Exp)
                        E.append(et)
                        if t == 1:
                            ds = dsp.tile([P, 512], f32, name="ds", tag=f"ds{t % 2}")
                            nc.vector.tensor_add(ds[:, :cw], E[0][:, :cw], E[1][:, :cw])
                            ds_prev = ds
                        elif t > 1:
                            ds = dsp.tile([P, 512], f32, name="ds", tag=f"ds{t % 2}")
                            nc.vector.tensor_add(ds[:, :cw], ds_prev[:, :cw], et[:, :cw])
                            ds_prev = ds

                    # O_T[s, o] = sum_t E[t, s] V_T[t, o] ;
                    # d[s] = ones^T ds (tiny fp32 matmul reduces over partitions)
                    for si in range(cw // P):
                        sl = slice(si * P, (si + 1) * P)
                        acc_o = po.tile([P, C], f32, name="acc_o", tag="po")
                        for t in range(TB):
                            nc.tensor.matmul(
                                acc_o[:, :C], E[t][:, sl], VT[t][:, :C],
                                start=(t == 0), stop=(t == TB - 1),
                            )
                        # d-matmul after the O chain: the DVE ds-chain finishes
                        # while the O matmuls stream, so the PE never stalls on it
                        acc_d = pp.tile([P, 512], f32, name="acc_d", tag="pp")
                        nc.tensor.matmul(
                            acc_d[:, :2], ds_prev[:, sl].bitcast(f32), ones32,
                            start=True, stop=True,
                        )
                        r = rp.tile([P, 1], f32, name="r", tag="r")
                        nc.vector.reciprocal(r, acc_d[:, :1])
                        ot = outp.tile([P, C], f32, name="ot", tag="ot")
                        nc.scalar.mul(ot, acc_o[:, :C], r)
                        nc.sync.dma_start(out_d[s0 // P + si], ot)
            imgp_cm.__exit__(None, None, None)

    nc.compile()
    return nc


def kernel(img_feat, text_feat, w_tp, b_tp, w_q, b_q, w_k, b_k, w_v, b_v, scale):
    from concourse.bass_utils import run_bass_kernel_spmd

    if "nc" not in _CACHE:
        _CACHE["nc"] = _build()
    nc = _CACHE["nc"]

    img_feat = np.ascontiguousarray(np.asarray(img_feat, dtype=np.float32))
    text_feat = np.ascontiguousarray(np.asarray(text_feat, dtype=np.float32))
    sc = float(np.asarray(scale).reshape(-1)[0])

    # host-side weight marshalling (fp64 intermediates):
    #  - fold w_tp into w_k / w_v (and b_tp into their biases)
    #  - fold `scale` into w_q / b_q
    #  - transpose everything to [c_in, c_out]
    w_tp64 = np.asarray(w_tp, np.float64)
    b_tp64 = np.asarray(b_tp, np.float64)
    wk_f = (np.asarray(w_k, np.float64) @ w_tp64)
    wv_f = (np.asarray(w_v, np.float64) @ w_tp64)
    bk_eff = (np.asarray(w_k, np.float64) @ b_tp64 + np.asarray(b_k, np.float64))
    bv_eff = (np.asarray(w_v, np.float64) @ b_tp64 + np.asarray(b_v, np.float64))

    wqT = np.ascontiguousarray((np.asarray(w_q, np.float32) * sc).T).reshape(CB, P, C)
    wkT = np.ascontiguousarray(wk_f.T.astype(np.float32)).reshape(CB, P, C)
    wvT = np.ascontiguousarray(wv_f.T.astype(np.float32)).reshape(CB, P, C)
    bq_h = (np.asarray(b_q, np.float32) * sc).reshape(CB, P, 1)
    bk_h = bk_eff.astype(np.float32).reshape(CB, P, 1)

    shared = {
        "wq": wqT, "wk": wkT, "wv": wvT,
        "bq": bq_h, "bk": bk_h,
        "ones": np.ones((P, 2), dtype=np.float32),
    }

    def chunk_major(x):  # [C, H, W] -> [NCH, CB, P, 512] (zero-padded tail)
        x = x.reshape(CB, P, S)
        out = np.zeros((len(CHUNKS), CB, P, 512), dtype=np.float32)
        for ci, (s0, cw) in enumerate(CHUNKS):
            out[ci, :, :, :cw] = x[:, :, s0:s0 + cw]
        return out

    in_maps = []
    for b in range(B):
        in_maps.append({
            "img": chunk_major(img_feat[b]),
            "text": chunk_major(text_feat[b]),
            **shared,
        })

    res = run_bass_kernel_spmd(nc, in_maps, core_ids=list(range(B)))
    _CACHE["last_result"] = res

    out = np.empty((B, C, H, W), dtype=np.float32)
    bv = bv_eff.astype(np.float32)
    for b in range(B):
        ot = res.results[b]["out"].reshape(S, C)          # [s, o]
        o_full = ot.T.reshape(C, H, W) + bv.reshape(C, 1, 1)
        out[b] = img_feat[b] + o_full
    return out


# revision 40
# speedup vs baseline: 1.0198x; 1.0198x over previous
"""Trainium2 Bass kernel for CrossModalAttention.

Problem: B=8, C=512, H=W=48 (S=2304 spatial), TC=512.
  tf = w_tp @ text + b_tp           (1x1 conv)
  Q  = w_q @ img + b_q ;  K = w_k @ tf + b_k ;  V = w_v @ tf + b_v
  attn = softmax_t(scale * Q^T K)   (full spatial cross attention)
  out  = img + attn @ V^T

Sharding: data-parallel over batch — one batch element per NeuronCore (8 cores).

Per-core device kernel (all big matmuls in float32r: full PE speed at N>=256,
~tf32 precision, fp32 PSUM accumulation):
  - the text projection is FOLDED on the host: K = (w_k w_tp) text + (w_k b_tp
    + b_k), V likewise => the tf intermediate never exists on device; K and
    V_T stream directly from text chunks.
  - weights are pre-transposed on host to [c_in, c_out] so no on-device
    transposes are needed anywhere.
  - `scale` is folded into w_q/b_q on the host (exact: softmax input identical).
  - scores are computed TRANSPOSED: st[t, s] = sum_o K[o,t] Q'[o,s] so that
    V_T[t, o] (computed directly transposed by swapping matmul operand roles)
    can consume exp(st) with no transposes. Softmax is computed without max
    subtraction (|scores| <= ~30 for this problem => exp is fp32-safe).
  - softmax denominator: VectorE tree-adds the 18 exp tiles (DVE is otherwise
    idle) and one tiny fp32 N=2 matmul per s-slice reduces over partitions;
    normalization is a per-partition reciprocal scale fused into the PSUM->SBUF
    copy of the output.
  - K's bias is applied on device (free per-partition ACT bias, and exactly
    cancels in softmax anyway); V's effective bias is added on the host
    (softmax rows sum to 1 so it contributes exactly +b_v_eff to out).
  - the kernel returns O_T[s, o]; host transposes and adds img_feat + b_v_eff.
  - text and img are DMA-streamed per 512-column chunk (chunk-major contiguous
    layout) over both HWDGE rings; weights go over the gpsimd SWDGE ring; a
    short N=512 warmup matmul burst holds the PE HAM clock at 2.4 GHz through
    the DMA-limited start.
"""

import numpy as np

B, C, S = 8, 512, 2304
H = W = 48
P = 128
CB = C // P     # 4 channel blocks
TB = S // P     # 18 key/value position blocks
# s-chunk widths (>=256 keeps float32r matmul at 1 cycle/row)
# small chunk first: only 512KB of text must land before the first matmul
CHUNKS = [(2048, 256), (0, 512), (512, 512), (1024, 512), (1536, 512)]

_CACHE = {}


def _build():
    """Trace + compile the per-core Bass kernel. Returns the compiled Bacc."""
    import concourse.bacc as bacc
    import concourse.tile as tile
    import concourse.mybir as mybir

    f32 = mybir.dt.float32
    f32r = mybir.dt.float32r
    AF = mybir.ActivationFunctionType

    nc = bacc.Bacc("TRN2", target_bir_lowering=False, debug=False)

    # chunk-major layout: [chunk, c_block, partition, 512] so each per-chunk
    # DMA is one fully contiguous 256KB block (max DMA bandwidth)
    NCH = len(CHUNKS)
    img_d = nc.dram_tensor("img", [NCH, CB, P, 512], f32r, kind="ExternalInput")
    text_d = nc.dram_tensor("text", [NCH, CB, P, 512], f32r, kind="ExternalInput")
    wq_d = nc.dram_tensor("wq", [CB, P, C], f32r, kind="ExternalInput")
    wk_d = nc.dram_tensor("wk", [CB, P, C], f32r, kind="ExternalInput")
    wv_d = nc.dram_tensor("wv", [CB, P, C], f32r, kind="ExternalInput")
    bq_d = nc.dram_tensor("bq", [CB, P, 1], f32, kind="ExternalInput")
    bk_d = nc.dram_tensor("bk", [CB, P, 1], f32, kind="ExternalInput")
    ones_d = nc.dram_tensor("ones", [P, 2], f32, kind="ExternalInput")
    out_d = nc.dram_tensor("out", [TB, P, C], f32, kind="ExternalOutput")

    # queue mode: later pools get fresh (ring) addresses instead of
    # immediately reusing earlier space => streaming DMAs don't WAR-stall
    with tile.TileContext(nc, pool_alloc_mode="queue") as tc:
        with (
            tc.tile_pool(name="pp", bufs=2, space="PSUM") as pp,
            tc.tile_pool(name="psc", bufs=3, space="PSUM") as psc,
            tc.tile_pool(name="po", bufs=3, space="PSUM") as po,
            # persistent SBUF
            tc.tile_pool(name="const", bufs=1) as constp,
            tc.tile_pool(name="wts", bufs=1) as wts,
            tc.tile_pool(name="kpool", bufs=1) as kpool,
            tc.tile_pool(name="vpool", bufs=1) as vpool,
        ):
            # PE warmup: full-width (N=512) matmuls on a memset tile keep the
            # PE array genuinely busy while the first text chunk DMAs in, so
            # the HAM clock gate reaches 8/8 (2.4 GHz) before real work starts.
            wu = constp.tile([P, 512], f32, name="wu", tag="wu")
            nc.vector.memset(wu, 0.0)
            wur = constp.tile([P, 512], f32r, name="wur", tag="wur")
            nc.scalar.activation(wur, wu, mybir.ActivationFunctionType.Copy)
            for i in range(24):
                acc_w = pp.tile([P, 512], f32, name="acc_w", tag="pp")
                nc.tensor.matmul(acc_w[:, :512], wur[:, :P], wur, start=True, stop=True)

            # const tiles; DMAs emitted after the first text chunk (their
            # issue latency must not delay the first streamed transfer)
            ones32 = constp.tile([P, 2], f32, name="ones32", tag="ones32")
            bq = [constp.tile([P, 1], f32, name=f"bq{c}", tag=f"bq{c}") for c in range(CB)]
            bk = [constp.tile([P, 1], f32, name=f"bk{c}", tag=f"bk{c}") for c in range(CB)]

            def load_consts():
                for c in range(CB):
                    nc.gpsimd.dma_start(bk[c], bk_d[c])
                    nc.gpsimd.dma_start(bq[c], bq_d[c])
                nc.gpsimd.dma_start(ones32, ones_d[:, :])

            # K and V weights are needed within the first few us of phase 1:
            # put them at the head of the two fast HWDGE rings (wk on sync,
            # wv on scalar), ahead of the text stream; wq/biases go over the
            # slower gpsimd SWDGE ring (needed much later)
            # wk split across all three DMA paths (first K group needs all 4
            # blocks); wv follows on SWDGE, landing before the first V_T group
            wk, wv, wq = [], [], []
            wk_eng = [nc.sync, nc.scalar, nc.sync, nc.scalar]
            for c in range(CB):
                t = wts.tile([P, C], f32r, name=f"wk{c}", tag=f"wk{c}")
                wk_eng[c].dma_start(t, wk_d[c])
                wk.append(t)
                wv.append(wts.tile([P, C], f32r, name=f"wv{c}", tag=f"wv{c}"))
                wq.append(wts.tile([P, C], f32r, name=f"wq{c}", tag=f"wq{c}"))
            for c in range(CB):
                nc.gpsimd.dma_start(wv[c], wv_d[c])

            K = [kpool.tile([P, S], f32r, name=f"k{c}", tag=f"k{c}") for c in range(CB)]
            VT = [vpool.tile([P, C], f32r, name=f"vt{t}", tag=f"vt{t}") for t in range(TB)]
            Qf = [kpool.tile([P, S], f32r, name=f"qf{o}", tag=f"qf{o}") for o in range(CB)]

            imgp_cm = tc.tile_pool(name="imgp", bufs=2)
            imgp = imgp_cm.__enter__()
            img_tiles = {}

            def load_img_chunk(ci, cw):
                tiles = []
                for c in range(CB):
                    it = imgp.tile([P, 512], f32r, name=f"imgc{c}", tag=f"imgc{c}")
                    eng = nc.sync if c % 2 == 0 else nc.scalar
                    eng.dma_start(it[:, :cw], img_d[ci, c][:, :cw])
                    tiles.append(it)
                img_tiles[ci] = tiles

            # ---- phase 1: K, V_T and Q, all streamed ----
            # K[o, t] = sum_c wk[c, o] text[c, t]  (+ b_k_eff)
            # V_T[t, o] = sum_c text[c, t] wv[c, o]  (b_v_eff added on host)
            # Q'[o, s] = sum_c wq[c, o] img[c, s]   (+ b_q_eff), one chunk
            # DELAYED so it acts as PE filler while the next text chunk lands
            def q_group(cj):
                s0j, cwj = CHUNKS[cj]
                img_c = img_tiles.pop(cj)
                for o in range(CB):
                    acc = pp.tile([P, 512], f32, name="acc_q", tag="pp")
                    for c in range(CB):
                        nc.tensor.matmul(
                            acc[:, :cwj],
                            wq[c][:, o * P:(o + 1) * P],
                            img_c[c][:, :cwj],
                            start=(c == 0), stop=(c == CB - 1),
                        )
                    nc.vector.tensor_scalar_add(
                        Qf[o][:, s0j:s0j + cwj], acc[:, :cwj], bq[o])

            with tc.tile_pool(name="textp", bufs=3) as textp:
                for ci, (s0, cw) in enumerate(CHUNKS):
                    text_c = []
                    for c in range(CB):
                        tt = textp.tile([P, 512], f32r, name=f"text{c}", tag=f"text{c}")
                        # split each chunk across both HWDGE rings (sync + scalar)
                        eng = nc.sync if c % 2 == 0 else nc.scalar
                        eng.dma_start(tt[:, :cw], text_d[ci, c][:, :cw])
                        text_c.append(tt)
                    if ci == 0:
                        load_consts()
                        for c in range(CB):
                            nc.gpsimd.dma_start(wq[c], wq_d[c])
                    else:
                        # img for the PREVIOUS chunk: consumed by q_group(ci-1)
                        # at the end of this iteration; keeps the ring head
                        # clear for wk + the first text chunks
                        load_img_chunk(ci - 1, CHUNKS[ci - 1][1])
                    for o in range(CB):
                        acc = pp.tile([P, 512], f32, name="acc_k", tag="pp")
                        for c in range(CB):
                            nc.tensor.matmul(
                                acc[:, :cw],
                                wk[c][:, o * P:(o + 1) * P],
                                text_c[c][:, :cw],
                                start=(c == 0), stop=(c == CB - 1),
                            )
                        nc.vector.tensor_scalar_add(
                            K[o][:, s0:s0 + cw], acc[:, :cw], bk[o])
                    for tl in range(cw // P):
                        t = s0 // P + tl
                        acc = pp.tile([P, 512], f32, name="acc_v", tag="pp")
                        for c in range(CB):
                            nc.tensor.matmul(
                                acc[:, :C],
                                text_c[c][:, tl * P:(tl + 1) * P],
                                wv[c][:, :C],
                                start=(c == 0), stop=(c == CB - 1),
                            )
                        nc.vector.tensor_copy(VT[t], acc[:, :C])
                    if ci > 0:
                        q_group(ci - 1)
                load_img_chunk(NCH - 1, CHUNKS[NCH - 1][1])
                q_group(NCH - 1)

            # ---- phase 2: attention, chunked over s ----
            with (
                tc.tile_pool(name="ep", bufs=1) as ep,
                tc.tile_pool(name="dsp", bufs=1) as dsp,
                tc.tile_pool(name="outp", bufs=3) as outp,
                tc.tile_pool(name="rp", bufs=2) as rp,
            ):
                for ci, (s0, cw) in enumerate(CHUNKS):
                    # E[t, s] = exp(sum_o K[o,t] Q'[o,s]);
                    # DVE tree-adds E tiles into ds (sum over t-blocks)
                    E = []
                    ds_prev = None
                    for t in range(TB):
                        acc = psc.tile([P, 512], f32, name="acc_sc", tag="psc")
                        for o in range(CB):
                            nc.tensor.matmul(
                                acc[:, :cw],
                                K[o][:, t * P:(t + 1) * P],
                                Qf[o][:, s0:s0 + cw],
                                start=(o == 0), stop=(o == CB - 1),
                            )
                        et = ep.tile([P, 512], f32r, name=f"e{t}", tag=f"e{t}")
                        nc.scalar.activation(et[:, :cw], acc[:, :cw], AF.Exp)
                        E.append(et)
                        if t == 1:
                            ds = dsp.tile([P, 512], f32, name="ds", tag=f"ds{t % 2}")
                            nc.vector.tensor_add(ds[:, :cw], E[0][:, :cw], E[1][:, :cw])
                            ds_prev = ds
                        elif t > 1:
                            ds = dsp.tile([P, 512], f32, name="ds", tag=f"ds{t % 2}")
                            nc.vector.tensor_add(ds[:, :cw], ds_prev[:, :cw], et[:, :cw])
                            ds_prev = ds

                    # O_T[s, o] = sum_t E[t, s] V_T[t, o] ;
                    # d[s] = ones^T ds (tiny fp32 matmul reduces over partitions)
                    for si in range(cw // P):
                        sl = slice(si * P, (si + 1) * P)
                        acc_o = po.tile([P, C], f32, name="acc_o", tag="po")
                        for t in range(TB):
                            nc.tensor.matmul(
                                acc_o[:, :C], E[t][:, sl], VT[t][:, :C],
                                start=(t == 0), stop=(t == TB - 1),
                            )
                        # d-matmul after the O chain: the DVE ds-chain finishes
                        # while the O matmuls stream, so the PE never stalls on it
                        acc_d = pp.tile([P, 512], f32, name="acc_d", tag="pp")
                        nc.tensor.matmul(
                            acc_d[:, :2], ds_prev[:, sl].bitcast(f32), ones32,
                            start=True, stop=True,
                        )
                        r = rp.tile([P, 1], f32, name="r", tag="r")
                        nc.vector.reciprocal(r, acc_d[:, :1])
                        ot = outp.tile([P, C], f32, name="ot", tag="ot")
                        nc.scalar.mul(ot, acc_o[:, :C], r)
                        nc.sync.dma_start(out_d[s0 // P + si], ot)
            imgp_cm.__exit__(None, None, None)

    nc.compile()
    return nc


def kernel(img_feat, text_feat, w_tp, b_tp, w_q, b_q, w_k, b_k, w_v, b_v, scale):
    from concourse.bass_utils import run_bass_kernel_spmd

    if "nc" not in _CACHE:
        _CACHE["nc"] = _build()
    nc = _CACHE["nc"]

    img_feat = np.ascontiguousarray(np.asarray(img_feat, dtype=np.float32))
    text_feat = np.ascontiguousarray(np.asarray(text_feat, dtype=np.float32))
    sc = float(np.asarray(scale).reshape(-1)[0])

    # host-side weight marshalling (fp64 intermediates):
    #  - fold w_tp into w_k / w_v (and b_tp into their biases)
    #  - fold `scale` into w_q / b_q
    #  - transpose everything to [c_in, c_out]
    w_tp64 = np.asarray(w_tp, np.float64)
    b_tp64 = np.asarray(b_tp, np.float64)
    wk_f = (np.asarray(w_k, np.float64) @ w_tp64)
    wv_f = (np.asarray(w_v, np.float64) @ w_tp64)
    bk_eff = (np.asarray(w_k, np.float64) @ b_tp64 + np.asarray(b_k, np.float64))
    bv_eff = (np.asarray(w_v, np.float64) @ b_tp64 + np.asarray(b_v, np.float64))

    wqT = np.ascontiguousarray((np.asarray(w_q, np.float32) * sc).T).reshape(CB, P, C)
    wkT = np.ascontiguousarray(wk_f.T.astype(np.float32)).reshape(CB, P, C)
    wvT = np.ascontiguousarray(wv_f.T.astype(np.float32)).reshape(CB, P, C)
    bq_h = (np.asarray(b_q, np.float32) * sc).reshape(CB, P, 1)
    bk_h = bk_eff.astype(np.float32).reshape(CB, P, 1)

    shared = {
        "wq": wqT, "wk": wkT, "wv": wvT,
        "bq": bq_h, "bk": bk_h,
        "ones": np.ones((P, 2), dtype=np.float32),
    }

    def chunk_major(x):  # [C, H, W] -> [NCH, CB, P, 512] (zero-padded tail)
        x = x.reshape(CB, P, S)
        out = np.zeros((len(CHUNKS), CB, P, 512), dtype=np.float32)
        for ci, (s0, cw) in enumerate(CHUNKS):
            out[ci, :, :, :cw] = x[:, :, s0:s0 + cw]
        return out

    in_maps = []
    for b in range(B):
        in_maps.append({
            "img": chunk_major(img_feat[b]),
            "text": chunk_major(text_feat[b]),
            **shared,
        })

    res = run_bass_kernel_spmd(nc, in_maps, core_ids=list(range(B)))
    _CACHE["last_result"] = res

    out = np.empty((B, C, H, W), dtype=np.float32)
    bv = bv_eff.astype(np.float32)
    for b in range(B):
        ot = res.results[b]["out"].reshape(S, C)          # [s, o]
        o_full = ot.T.reshape(C, H, W) + bv.reshape(C, 1, 1)
        out[b] = img_feat[b] + o_full
    return out


# revision 41
# speedup vs baseline: 1.0347x; 1.0146x over previous
"""Trainium2 Bass kernel for CrossModalAttention.

Problem: B=8, C=512, H=W=48 (S=2304 spatial), TC=512.
  tf = w_tp @ text + b_tp           (1x1 conv)
  Q  = w_q @ img + b_q ;  K = w_k @ tf + b_k ;  V = w_v @ tf + b_v
  attn = softmax_t(scale * Q^T K)   (full spatial cross attention)
  out  = img + attn @ V^T

Sharding: data-parallel over batch — one batch element per NeuronCore (8 cores).

Per-core device kernel (all big matmuls in float32r: full PE speed at N>=256,
~tf32 precision, fp32 PSUM accumulation):
  - the text projection is FOLDED on the host: K = (w_k w_tp) text + (w_k b_tp
    + b_k), V likewise => the tf intermediate never exists on device; K and
    V_T stream directly from text chunks.
  - weights are pre-transposed on host to [c_in, c_out] so no on-device
    transposes are needed anywhere.
  - `scale` is folded into w_q/b_q on the host (exact: softmax input identical).
  - scores are computed TRANSPOSED: st[t, s] = sum_o K[o,t] Q'[o,s] so that
    V_T[t, o] (computed directly transposed by swapping matmul operand roles)
    can consume exp(st) with no transposes. Softmax is computed without max
    subtraction (|scores| <= ~30 for this problem => exp is fp32-safe).
  - softmax denominator: VectorE tree-adds the 18 exp tiles (DVE is otherwise
    idle) and one tiny fp32 N=2 matmul per s-slice reduces over partitions;
    normalization is a per-partition reciprocal scale fused into the PSUM->SBUF
    copy of the output.
  - K's bias is applied on device (free per-partition ACT bias, and exactly
    cancels in softmax anyway); V's effective bias is added on the host
    (softmax rows sum to 1 so it contributes exactly +b_v_eff to out).
  - the kernel returns O_T[s, o]; host transposes and adds img_feat + b_v_eff.
  - text and img are DMA-streamed per 512-column chunk (chunk-major contiguous
    layout) over both HWDGE rings; weights go over the gpsimd SWDGE ring; a
    short N=512 warmup matmul burst holds the PE HAM clock at 2.4 GHz through
    the DMA-limited start.
"""

import numpy as np

B, C, S = 8, 512, 2304
H = W = 48
P = 128
CB = C // P     # 4 channel blocks
TB = S // P     # 18 key/value position blocks
# s-chunk widths (>=256 keeps float32r matmul at 1 cycle/row)
# small chunk first: only 512KB of text must land before the first matmul
CHUNKS = [(2048, 256), (0, 512), (512, 512), (1024, 512), (1536, 512)]

_CACHE = {}


def _build():
    """Trace + compile the per-core Bass kernel. Returns the compiled Bacc."""
    import concourse.bacc as bacc
    import concourse.tile as tile
    import concourse.mybir as mybir

    f32 = mybir.dt.float32
    f32r = mybir.dt.float32r
    AF = mybir.ActivationFunctionType

    nc = bacc.Bacc("TRN2", target_bir_lowering=False, debug=False)

    # chunk-major layout: [chunk, c_block, partition, 512] so each per-chunk
    # DMA is one fully contiguous 256KB block (max DMA bandwidth)
    NCH = len(CHUNKS)
    img_d = nc.dram_tensor("img", [NCH, CB, P, 512], f32r, kind="ExternalInput")
    text_d = nc.dram_tensor("text", [NCH, CB, P, 512], f32r, kind="ExternalInput")
    wq_d = nc.dram_tensor("wq", [CB, P, C], f32r, kind="ExternalInput")
    wk_d = nc.dram_tensor("wk", [CB, P, C], f32r, kind="ExternalInput")
    wv_d = nc.dram_tensor("wv", [CB, P, C], f32r, kind="ExternalInput")
    bq_d = nc.dram_tensor("bq", [CB, P, 1], f32, kind="ExternalInput")
    bk_d = nc.dram_tensor("bk", [CB, P, 1], f32, kind="ExternalInput")
    ones_d = nc.dram_tensor("ones", [P, 2], f32, kind="ExternalInput")
    out_d = nc.dram_tensor("out", [TB, P, C], f32, kind="ExternalOutput")

    # queue mode: later pools get fresh (ring) addresses instead of
    # immediately reusing earlier space => streaming DMAs don't WAR-stall
    with tile.TileContext(nc, pool_alloc_mode="queue") as tc:
        with (
            tc.tile_pool(name="pp", bufs=3, space="PSUM") as pp,
            tc.tile_pool(name="psc", bufs=3, space="PSUM") as psc,
            tc.tile_pool(name="po", bufs=2, space="PSUM") as po,
            # persistent SBUF
            tc.tile_pool(name="const", bufs=1) as constp,
            tc.tile_pool(name="wts", bufs=1) as wts,
            tc.tile_pool(name="kpool", bufs=1) as kpool,
            tc.tile_pool(name="vpool", bufs=1) as vpool,
        ):
            # PE warmup: full-width (N=512) matmuls on a memset tile keep the
            # PE array genuinely busy while the first text chunk DMAs in, so
            # the HAM clock gate reaches 8/8 (2.4 GHz) before real work starts.
            wu = constp.tile([P, 512], f32, name="wu", tag="wu")
            nc.vector.memset(wu, 0.0)
            wur = constp.tile([P, 512], f32r, name="wur", tag="wur")
            nc.scalar.activation(wur, wu, mybir.ActivationFunctionType.Copy)
            for i in range(31):
                acc_w = pp.tile([P, 512], f32, name="acc_w", tag="pp")
                nc.tensor.matmul(acc_w[:, :512], wur[:, :P], wur, start=True, stop=True)

            # const tiles; DMAs emitted after the first text chunk (their
            # issue latency must not delay the first streamed transfer)
            ones32 = constp.tile([P, 2], f32, name="ones32", tag="ones32")
            bq = [constp.tile([P, 1], f32, name=f"bq{c}", tag=f"bq{c}") for c in range(CB)]
            bk = [constp.tile([P, 1], f32, name=f"bk{c}", tag=f"bk{c}") for c in range(CB)]

            def load_consts():
                for c in range(CB):
                    nc.gpsimd.dma_start(bk[c], bk_d[c])
                    nc.gpsimd.dma_start(bq[c], bq_d[c])
                nc.gpsimd.dma_start(ones32, ones_d[:, :])

            # K and V weights are needed within the first few us of phase 1:
            # put them at the head of the two fast HWDGE rings (wk on sync,
            # wv on scalar), ahead of the text stream; wq/biases go over the
            # slower gpsimd SWDGE ring (needed much later)
            # wk split across all three DMA paths (first K group needs all 4
            # blocks); wv follows on SWDGE, landing before the first V_T group
            wk, wv, wq = [], [], []
            wk_eng = [nc.sync, nc.scalar, nc.sync, nc.scalar]
            for c in range(CB):
                t = wts.tile([P, C], f32r, name=f"wk{c}", tag=f"wk{c}")
                wk_eng[c].dma_start(t, wk_d[c])
                wk.append(t)
                wv.append(wts.tile([P, C], f32r, name=f"wv{c}", tag=f"wv{c}"))
                wq.append(wts.tile([P, C], f32r, name=f"wq{c}", tag=f"wq{c}"))
            for c in range(CB):
                nc.gpsimd.dma_start(wv[c], wv_d[c])

            K = [kpool.tile([P, S], f32r, name=f"k{c}", tag=f"k{c}") for c in range(CB)]
            VT = [vpool.tile([P, C], f32r, name=f"vt{t}", tag=f"vt{t}") for t in range(TB)]
            Qf = [kpool.tile([P, S], f32r, name=f"qf{o}", tag=f"qf{o}") for o in range(CB)]

            imgp_cm = tc.tile_pool(name="imgp", bufs=2)
            imgp = imgp_cm.__enter__()
            img_tiles = {}

            def load_img_chunk(ci, cw):
                tiles = []
                for c in range(CB):
                    it = imgp.tile([P, 512], f32r, name=f"imgc{c}", tag=f"imgc{c}")
                    eng = nc.sync if c % 2 == 0 else nc.scalar
                    eng.dma_start(it[:, :cw], img_d[ci, c][:, :cw])
                    tiles.append(it)
                img_tiles[ci] = tiles

            # ---- phase 1: K, V_T and Q, all streamed ----
            # K[o, t] = sum_c wk[c, o] text[c, t]  (+ b_k_eff)
            # V_T[t, o] = sum_c text[c, t] wv[c, o]  (b_v_eff added on host)
            # Q'[o, s] = sum_c wq[c, o] img[c, s]   (+ b_q_eff), one chunk
            # DELAYED so it acts as PE filler while the next text chunk lands
            def q_group(cj):
                s0j, cwj = CHUNKS[cj]
                img_c = img_tiles.pop(cj)
                for o in range(CB):
                    acc = pp.tile([P, 512], f32, name="acc_q", tag="pp")
                    for c in range(CB):
                        nc.tensor.matmul(
                            acc[:, :cwj],
                            wq[c][:, o * P:(o + 1) * P],
                            img_c[c][:, :cwj],
                            start=(c == 0), stop=(c == CB - 1),
                        )
                    nc.vector.tensor_scalar_add(
                        Qf[o][:, s0j:s0j + cwj], acc[:, :cwj], bq[o])

            with tc.tile_pool(name="textp", bufs=3) as textp:
                for ci, (s0, cw) in enumerate(CHUNKS):
                    text_c = []
                    for c in range(CB):
                        tt = textp.tile([P, 512], f32r, name=f"text{c}", tag=f"text{c}")
                        # split each chunk across both HWDGE rings (sync + scalar)
                        eng = nc.sync if c % 2 == 0 else nc.scalar
                        eng.dma_start(tt[:, :cw], text_d[ci, c][:, :cw])
                        text_c.append(tt)
                    if ci == 0:
                        load_consts()
                        for c in range(CB):
                            nc.gpsimd.dma_start(wq[c], wq_d[c])
                    else:
                        # img for the PREVIOUS chunk: consumed by q_group(ci-1)
                        # at the end of this iteration; keeps the ring head
                        # clear for wk + the first text chunks
                        load_img_chunk(ci - 1, CHUNKS[ci - 1][1])
                    for o in range(CB):
                        acc = pp.tile([P, 512], f32, name="acc_k", tag="pp")
                        for c in range(CB):
                            nc.tensor.matmul(
                                acc[:, :cw],
                                wk[c][:, o * P:(o + 1) * P],
                                text_c[c][:, :cw],
                                start=(c == 0), stop=(c == CB - 1),
                            )
                        nc.vector.tensor_scalar_add(
                            K[o][:, s0:s0 + cw], acc[:, :cw], bk[o])
                    for tl in range(cw // P):
                        t = s0 // P + tl
                        acc = pp.tile([P, 512], f32, name="acc_v", tag="pp")
                        for c in range(CB):
                            nc.tensor.matmul(
                                acc[:, :C],
                                text_c[c][:, tl * P:(tl + 1) * P],
                                wv[c][:, :C],
                                start=(c == 0), stop=(c == CB - 1),
                            )
                        nc.vector.tensor_copy(VT[t], acc[:, :C])
                    if ci > 0:
                        q_group(ci - 1)
                load_img_chunk(NCH - 1, CHUNKS[NCH - 1][1])
                q_group(NCH - 1)

            # ---- phase 2: attention, chunked over s ----
            with (
                tc.tile_pool(name="ep", bufs=1) as ep,
                tc.tile_pool(name="dsp", bufs=1) as dsp,
                tc.tile_pool(name="outp", bufs=3) as outp,
                tc.tile_pool(name="rp", bufs=2) as rp,
            ):
                for ci, (s0, cw) in enumerate(CHUNKS):
                    # E[t, s] = exp(sum_o K[o,t] Q'[o,s]);
                    # DVE tree-adds E tiles into ds (sum over t-blocks)
                    E = []
                    ds_prev = None
                    for t in range(TB):
                        acc = psc.tile([P, 512], f32, name="acc_sc", tag="psc")
                        for o in range(CB):
                            nc.tensor.matmul(
                                acc[:, :cw],
                                K[o][:, t * P:(t + 1) * P],
                                Qf[o][:, s0:s0 + cw],
                                start=(o == 0), stop=(o == CB - 1),
                            )
                        et = ep.tile([P, 512], f32r, name=f"e{t}", tag=f"e{t}")
                        nc.scalar.activation(et[:, :cw], acc[:, :cw], AF.Exp)
                        E.append(et)
                        if t == 1:
                            ds = dsp.tile([P, 512], f32, name="ds", tag=f"ds{t % 2}")
                            nc.vector.tensor_add(ds[:, :cw], E[0][:, :cw], E[1][:, :cw])
                            ds_prev = ds
                        elif t > 1:
                            ds = dsp.tile([P, 512], f32, name="ds", tag=f"ds{t % 2}")
                            nc.vector.tensor_add(ds[:, :cw], ds_prev[:, :cw], et[:, :cw])
                            ds_prev = ds

                    # O_T[s, o] = sum_t E[t, s] V_T[t, o] ;
                    # d[s] = ones^T ds (tiny fp32 matmul reduces over partitions)
                    for si in range(cw // P):
                        sl = slice(si * P, (si + 1) * P)
                        acc_o = po.tile([P, C], f32, name="acc_o", tag="po")
                        for t in range(TB):
                            nc.tensor.matmul(
                                acc_o[:, :C], E[t][:, sl], VT[t][:, :C],
                                start=(t == 0), stop=(t == TB - 1),
                            )
                        # d-matmul after the O chain: the DVE ds-chain finishes
                        # while the O matmuls stream, so the PE never stalls on it
                        acc_d = pp.tile([P, 512], f32, name="acc_d", tag="pp")
                        nc.tensor.matmul(
                            acc_d[:, :2], ds_prev[:, sl].bitcast(f32), ones32,
                            start=True, stop=True,
                        )
                        r = rp.tile([P, 1], f32, name="r", tag="r")
                        nc.vector.reciprocal(r, acc_d[:, :1])
                        ot = outp.tile([P, C], f32, name="ot", tag="ot")
                        nc.scalar.mul(ot, acc_o[:, :C], r)
                        nc.sync.dma_start(out_d[s0 // P + si], ot)
            imgp_cm.__exit__(None, None, None)

    nc.compile()
    return nc


def kernel(img_feat, text_feat, w_tp, b_tp, w_q, b_q, w_k, b_k, w_v, b_v, scale):
    from concourse.bass_utils import run_bass_kernel_spmd

    if "nc" not in _CACHE:
        _CACHE["nc"] = _build()
    nc = _CACHE["nc"]

    img_feat = np.ascontiguousarray(np.asarray(img_feat, dtype=np.float32))
    text_feat = np.ascontiguousarray(np.asarray(text_feat, dtype=np.float32))
    sc = float(np.asarray(scale).reshape(-1)[0])

    # host-side weight marshalling (fp64 intermediates):
    #  - fold w_tp into w_k / w_v (and b_tp into their biases)
    #  - fold `scale` into w_q / b_q
    #  - transpose everything to [c_in, c_out]
    w_tp64 = np.asarray(w_tp, np.float64)
    b_tp64 = np.asarray(b_tp, np.float64)
    wk_f = (np.asarray(w_k, np.float64) @ w_tp64)
    wv_f = (np.asarray(w_v, np.float64) @ w_tp64)
    bk_eff = (np.asarray(w_k, np.float64) @ b_tp64 + np.asarray(b_k, np.float64))
    bv_eff = (np.asarray(w_v, np.float64) @ b_tp64 + np.asarray(b_v, np.float64))

    wqT = np.ascontiguousarray((np.asarray(w_q, np.float32) * sc).T).reshape(CB, P, C)
    wkT = np.ascontiguousarray(wk_f.T.astype(np.float32)).reshape(CB, P, C)
    wvT = np.ascontiguousarray(wv_f.T.astype(np.float32)).reshape(CB, P, C)
    bq_h = (np.asarray(b_q, np.float32) * sc).reshape(CB, P, 1)
    bk_h = bk_eff.astype(np.float32).reshape(CB, P, 1)

    shared = {
        "wq": wqT, "wk": wkT, "wv": wvT,
        "bq": bq_h, "bk": bk_h,
        "ones": np.ones((P, 2), dtype=np.float32),
    }

    def chunk_major(x):  # [C, H, W] -> [NCH, CB, P, 512] (zero-padded tail)
        x = x.reshape(CB, P, S)
        out = np.zeros((len(CHUNKS), CB, P, 512), dtype=np.float32)
        for ci, (s0, cw) in enumerate(CHUNKS):
            out[ci, :, :, :cw] = x[:, :, s0:s0 + cw]
        return out

    in_maps = []
    for b in range(B):
        in_maps.append({
            "img": chunk_major(img_feat[b]),
            "text": chunk_major(text_feat[b]),
            **shared,
        })

    res = run_bass_kernel_spmd(nc, in_maps, core_ids=list(range(B)))
    _CACHE["last_result"] = res

    out = np.empty((B, C, H, W), dtype=np.float32)
    bv = bv_eff.astype(np.float32)
    for b in range(B):
        ot = res.results[b]["out"].reshape(S, C)          # [s, o]
        o_full = ot.T.reshape(C, H, W) + bv.reshape(C, 1, 1)
        out[b] = img_feat[b] + o_full
    return out
